# revision 27
# baseline (speedup 1.0000x reference)
"""Trainium2 Bass kernel for a ViT-style attention block + classifier head.

Reference computation (per batch b of 4, N=2048 tokens, C=768, 12 heads x 64):
    qkv  = x @ w_qkv                         [B,N,3C]
    attn = softmax(q k^T / 8)                per head
    out  = (attn @ v) reassembled            [B,N,C]
    out  = out @ w_proj + b_proj
    out  = out @ w_head + b_head             [B,N,1000]
    return max over N                        [B,1000]

Sharding: 8 cores = 4 batches x 2 query-halves (1024 queries each).
Each core computes K/V for its full batch, attention for its query half,
then a fused (w_proj @ w_head) classifier matmul and a local max over its
1024 queries -> [128,1000] per core; host reduces partitions + pairs and
adds the fused bias (max is invariant to per-row constants).

v3 design (vs the 242us v2 baseline) — attack all three busy engines:

* All projections (q/k/v) run in fp8e4m3 with MatmulPerfMode.DoubleRow:
  x and w_qkv are cast to fp8 host-side (w scaled by 32 to clear the
  subnormal range; compensated in the exp scale and classifier weight),
  packed in a [chunk, slot, partition] layout so each 256-deep
  contraction is 1 DR matmul (3 per 768 instead of 6 bf16 chunks).
  PE cost of the projections drops 4x (77us -> 19us).

* attn@v also runs DR: the exp stream writes fp8 e-tiles [128, 2, 1024]
  (kc-pair slots), v is produced once for all heads into [128, 2, 780]
  pair tiles (64 cols + ones col per head), so each (head, qb) output
  accumulates over 8 DR matmuls instead of 16 bf16 ones (41.6 -> 10.4us).
  fp8 perturbs only softmax weights and v; errors average across ~1.5k
  effective keys (measured end-to-end ~1.1e-2 vs the 2e-2 gate).

* The exp stream itself is split across TWO engines: ScalarE runs the
  Exp activation for ~10 kc of each head; the DVE runs a Schraudolph
  fast-exp for the other ~6 — one tensor_scalar per [128,1024] tile
  computing round(s * 8*log2e*scale + 55.55) into uint8, whose bits ARE
  fp8e4m3 exp(s*scale) to within +-8% (the piecewise-linear-in-mantissa
  exp approximation; bias cancels in softmax, noise averages out).
  HW CONSTRAINT (found by device bisection): an e2 tile whose two kc
  slots are written by DIFFERENT engines wedges the exec unit
  (NRT_EXEC_UNIT_UNRECOVERABLE), so DVE_KCS must hold whole kc-pairs.

* PSUM is one shared 3-deep ring of [128,1024]f32 tiles (6 banks) for
  scores AND the projection/v/pa evacuation slabs, + 1 bank for the
  4-rotation attn@v accumulator + 1 for the transpose stage.  The
  3-deep ring lets scores(kc+2) prefetch while the two exp engines
  drain kc and kc+1; batched 4-wide reciprocals, window-end-deferred
  transposes and a split lead-in DMA ([xTq|pair-0 w] first) keep the
  DVE queue from head-of-line blocking the stream.

* Classifier stays bf16 (fp8 dot-product noise does not average out
  there); split-partial (PA) + identity-refold tail, local max on the
  idle scores ring.

Measured (TimelineSim on the exact per-core program): 201.2 us/core
(v2 baseline: 241.8), rel err vs the fp32 reference 1.05e-2 on real
TRN2 via the axon relay.  ScalarE ~137us busy, DVE ~140us busy are the
co-critical engines; PE ~100us.
"""

import math
import sys

for _p in ("/opt/trn_rl_repo", "/root/.axon_site/_ro/trn_rl_repo"):
    if _p not in sys.path:
        sys.path.append(_p)

import numpy as np
import ml_dtypes

import concourse.bacc as bacc
import concourse.mybir as mybir
from concourse.tile import TileContext
from concourse.bass_utils import run_bass_kernel_spmd
from concourse.masks import make_identity

BF16 = mybir.dt.bfloat16
F32 = mybir.dt.float32
FP8 = mybir.dt.float8e4
U8 = mybir.dt.uint8
DR = mybir.MatmulPerfMode.DoubleRow

B, N, C = 4, 2048, 768
HEADS, HD = 12, 64
NUM_CLASSES = 1000
SCALE = HD ** (-0.5)
WS = 32.0                    # host-side fp8 weight scale
EXPSC = SCALE / (WS * WS)    # exp input scale (q,k each carry a WS)

NQ = 1024           # queries per core
KC = N // 128       # 16 key chunks
CC = 3              # DR contraction chunks (256 rows each)
PAIRS = HEADS // 2
NCLS = NUM_CLASSES
NJ = KC // 2        # 8 kc-pairs (DR attn@v contraction steps)

# Schraudolph fast-exp constants: uint8 bits = round(s*A_SCH + B_SCH)
# reinterpreted as fp8e4m3 ~= exp(s*EXPSC).
A_SCH = EXPSC * 8.0 / math.log(2.0)
B_SCH = 55.55

# kc values whose exp runs on the DVE (per head); the rest run on ScalarE.
# Isolated (non-adjacent) kc keep the 2-bank scores ring from coupling the
# two exp streams: Act only waits out the ~0.2us rate difference, not a
# whole DVE unit.
# PAIR-granular only: an e2 tile written by both engines (one slot each)
# crashes the exec unit on hardware — every kc-pair's two slots must come
# from ONE engine.
DVE_KCS = {h: ((2, 3, 8, 9, 12, 13) if h >= 1 else ()) for h in range(HEADS)}
DVE_KCS[1] = (8, 9, 12, 13)   # head 1's DVE is busy with the v-production

_CACHE = {}


def _build():
    nc = bacc.Bacc("TRN2", target_bir_lowering=False)

    # Host-packed fp8 inputs in DR row layout (row = c*256 + s*128 + p):
    #   qkp: [xT query half | w_q | w_k] per row — the lead-in working set
    #   xTk: key half of xT;  wv: w_v;  wf: fused classifier weight (bf16,
    #   plain c*128+p rows).
    qkp_d = nc.dram_tensor("qkp", [2 * CC * 128, NQ + 256], FP8, kind="ExternalInput")
    wrest_d = nc.dram_tensor("wrest", [2 * CC * 128, 2 * C - 256], FP8, kind="ExternalInput")
    xTk_d = nc.dram_tensor("xTk", [2 * CC * 128, NQ], FP8, kind="ExternalInput")
    wv_d = nc.dram_tensor("wv", [2 * CC * 128, C], FP8, kind="ExternalInput")
    wf_d = nc.dram_tensor("wf", [C, NCLS], BF16, kind="ExternalInput")
    out_d = nc.dram_tensor("out", [128, NCLS], BF16, kind="ExternalOutput")

    EXP = mybir.ActivationFunctionType.Exp

    with TileContext(nc) as tc:
        with (
            tc.tile_pool(name="wpool", bufs=1) as wpool,
            tc.tile_pool(name="xpool", bufs=1) as xpool,
            tc.tile_pool(name="stgp", bufs=1) as stgp,    # fp8 q/k DR tiles
            tc.tile_pool(name="vp", bufs=1) as vp,        # v65 pair tiles
            tc.tile_pool(name="ep", bufs=1) as ep,        # fp8 e2 tiles (2 head-sets)
            tc.tile_pool(name="stp", bufs=1) as stp,      # normalized [q, 2hd] staging
            tc.tile_pool(name="outp", bufs=1) as outp,
            tc.tile_pool(name="smallp", bufs=1) as smallp,
            tc.tile_pool(name="lgp", bufs=1) as lgp,
            # PSUM: one shared 3-deep ring of [128,1024]f32 tiles (6 banks)
            # serving scores AND the proj/v/pa evacuation slabs + av ring (1)
            # + transpose stage (1).  3-deep decouples the two exp engines:
            # scores(kc+2) prefetches while exp(kc)/exp(kc+1) drain.
            tc.tile_pool(name="sps", bufs=1, space="PSUM") as sps,
            tc.tile_pool(name="avps", bufs=1, space="PSUM") as avps,
            tc.tile_pool(name="tpps", bufs=1, space="PSUM") as tpps,
        ):
            ident = smallp.tile([128, 128], BF16, name="ident")

            # ---- persistent inputs ----
            xw = xpool.tile([128, CC, 2, NQ + 256], FP8, tag="xw", name="xw_sb")
            wrest = xpool.tile([128, CC, 2, 2 * C - 256], FP8, tag="wrest", name="wrest_sb")
            xTk = xpool.tile([128, CC, 2, NQ], FP8, tag="xTk", name="xTk_sb")
            wv_sb = wpool.tile([128, CC, 2, C], FP8, tag="wv", name="wv_sb")
            wf = wpool.tile([128, 2 * CC, NCLS], BF16, tag="wf", name="wf_sb")

            def xslice(c, n0, nw):
                """[128, 2, nw] DR chunk of xT columns [n0, n0+nw)."""
                assert n0 // NQ == (n0 + nw - 1) // NQ
                if n0 < NQ:
                    return xw[:, c, :, n0:n0 + nw]
                return xTk[:, c, :, n0 - NQ:n0 - NQ + nw]

            def wcol(c, which, p):
                """DR weight block [128, 2, 128] for pair p's q or k."""
                if p == 0:
                    o = NQ + (0 if which == "q" else 128)
                    return xw[:, c, :, o:o + 128]
                o = (p - 1) * 128 + (0 if which == "q" else 5 * 128)
                return wrest[:, c, :, o:o + 128]

            def load_inputs_phase(phase):
                if phase == 0:
                    # qkp in (c, s) row groups of [2,1,2,1] alternating the
                    # two HW queues; DR matmul c needs groups up to 2c+1 so
                    # the projection pipeline starts after 1/3 of the bytes.
                    o = 0
                    for i, g in enumerate((2, 1, 2, 1)):
                        eng = nc.sync if i % 2 == 0 else nc.scalar
                        eng.dma_start(
                            out=xw[:].rearrange("p a b n -> p (a b) n")[:, o:o + g, :],
                            in_=qkp_d[o * 128:(o + g) * 128, :].rearrange(
                                "(a p) n -> p a n", p=128))
                        o += g
                elif phase == 1:        # weights for pairs 1-5
                    nc.scalar.dma_start(
                        out=wrest[:].rearrange("p a b n -> p (a b) n"),
                        in_=wrest_d[:].rearrange("(a p) n -> p a n", p=128))
                elif phase == 2:        # key half of xT
                    nc.scalar.dma_start(
                        out=xTk[:].rearrange("p a b n -> p (a b) n"),
                        in_=xTk_d[:].rearrange("(a p) n -> p a n", p=128))
                elif phase == 3:        # w_v
                    nc.scalar.dma_start(
                        out=wv_sb[:].rearrange("p a b n -> p (a b) n"),
                        in_=wv_d[:].rearrange("(a p) n -> p a n", p=128))
                elif phase == 4:        # classifier weight (tail only)
                    nc.scalar.dma_start(
                        out=wf[:],
                        in_=wf_d[:].rearrange("(a p) n -> p a n", p=128))

            # fp8 q/k per pair in scores-DR layout [128, 2, N]: slot 0 holds
            # the real qT/kT (pair's two heads stacked on partitions, exactly
            # the projection PSUM layout -> lane-local cast), slot 1 zeroed.
            qP = {}
            kP = {}

            def alloc_qkP(p):
                if p not in qP:
                    qP[p] = stgp.tile([128, 2, NQ], FP8, tag="qP", name="qP_sb", bufs=2)
                    nc.gpsimd.memset(qP[p][:, 1, :], 0.0)
                if p not in kP:
                    kP[p] = stgp.tile([128, 2, N], FP8, tag="kP", name="kP_sb", bufs=2)
                    nc.gpsimd.memset(kP[p][:, 1, :], 0.0)

            def qk_unit(p, which, n0, nw=1024):
                """One 1024-col slab of pair p's q or k projection: 3 DR
                matmuls (bank-aligned 512 halves) -> lane-local fp8 cast
                into the scores-DR tile."""
                alloc_qkP(p)
                dst = qP[p] if which == "q" else kP[p]
                ps = sps.tile([128, NQ], F32, tag="s", name="s", bufs=3)
                for c in range(CC):
                    for o in range(0, nw, 512):
                        nc.tensor.matmul(
                            ps[:, o:o + 512], lhsT=wcol(c, which, p),
                            rhs=xslice(c, n0 + o, 512),
                            start=(c == 0), stop=(c == CC - 1), perf_mode=DR)
                if p == 0 and which == "k" and n0 < NQ:
                    # pair 0's first k cast rides the still-idle Activation
                    # queue so the lead-in cast chain runs two-wide
                    nc.scalar.copy(out=dst[:, 0, n0:n0 + nw], in_=ps[:, 0:nw])
                else:
                    nc.vector.tensor_copy(out=dst[:, 0, n0:n0 + nw], in_=ps[:, 0:nw])

            # v65 pair tiles: v65[j][:, s, :] holds keys of kc=2j+s, columns
            # h*65..h*65+64 = head h's v (+ ones col at h*65+64).
            v65 = [vp.tile([128, 2, HEADS * (HD + 1)], FP8, tag="v65",
                           name="v65_sb", bufs=NJ)
                   for _ in range(NJ)]
            v65_ones = [False] * NJ

            def v_unit(kc):
                """v for keys of chunk kc, all 12 heads (bank-aligned
                512/256 matmul halves, one strided evacuation)."""
                j, s = kc // 2, kc % 2
                ps = sps.tile([128, NQ], F32, tag="s", name="s", bufs=3)
                for c in range(CC):
                    for o, w in ((0, 512), (512, 256)):
                        nc.tensor.matmul(
                            ps[:, o:o + w], lhsT=xslice(c, kc * 128, 128),
                            rhs=wv_sb[:, c, :, o:o + w],
                            start=(c == 0), stop=(c == CC - 1), perf_mode=DR)
                vdst = v65[j][:, s, :].rearrange("p (h d) -> p h d", d=HD + 1)
                if not v65_ones[j]:
                    v65_ones[j] = True
                    od = v65[j][:].rearrange("p s (h d) -> p s h d", d=HD + 1)
                    nc.gpsimd.memset(od[:, :, :, HD:HD + 1], 1.0)
                nc.vector.tensor_copy(
                    out=vdst[:, :, 0:HD],
                    in_=ps[:, 0:C].rearrange("p (h d) -> p h d", d=HD))

            e_tiles = {}      # h -> [128, 2, NQ] fp8 tile list per kc-pair j
            st_tiles = {}     # p -> [8 staging tiles]
            tp_tiles = {}
            tail_r = {}

            def av_mm(h, qb):
                """attn@v matmul chain for head h, query block qb:
                out[q,65] accumulated over 8 DR kc-pair matmuls."""
                if "av" not in tp_tiles:
                    tp_tiles["av"] = avps.tile([128, 4, HD + 1], F32, name="avt", bufs=1)
                av = tp_tiles["av"][:, qb % 4, :]
                es = e_tiles[h]
                for j in range(NJ):
                    nc.tensor.matmul(
                        av[:], lhsT=es[j][:, :, qb * 128:(qb + 1) * 128],
                        rhs=v65[j][:, :, h * (HD + 1):(h + 1) * (HD + 1)],
                        start=(j == 0), stop=(j == NJ - 1), perf_mode=DR)

            def av_norm(h, g):
                """normalize query blocks 4g..4g+3 of head h: one batched
                4-wide reciprocal over the avt rotation, then 4 scaled
                evacuations into the transpose staging tiles."""
                p, hh = h // 2, h % 2
                avt = tp_tiles["av"]
                r4 = smallp.tile([128, 4], F32, tag="r", name="r", bufs=2)
                nc.vector.reciprocal_approx_fast(out=r4[:], in_=avt[:, :, HD])
                for qb in range(4 * g, 4 * g + 4):
                    av = avt[:, qb % 4, :]
                    if hh == 0:
                        if p not in st_tiles:
                            st_tiles[p] = []
                        st = stp.tile([128, 128], BF16, tag="st", name="st", bufs=16)
                        st_tiles[p].append(st)
                    else:
                        st = st_tiles[p][qb]
                    nc.vector.tensor_scalar_mul(
                        out=st[:, 64 * hh:64 * hh + 64], in0=av[:, 0:HD],
                        scalar1=r4[:, qb % 4:qb % 4 + 1])

            def av_unit(h, qb):
                """tail-only (head 11): av chain + per-qb normalize fully
                on the then-idle ScalarE (one-time switch to the
                reciprocal act table keeps the DVE out of the chain; it
                only runs the final maxes)."""
                p, hh = h // 2, h % 2
                av_mm(h, qb)
                av = tp_tiles["av"][:, qb % 4, :]
                r = smallp.tile([128, 1], F32, tag="r1", name="r1", bufs=4)
                nc.vector.reciprocal_approx_fast(out=r[:], in_=av[:, HD:HD + 1])
                st = st_tiles[p][qb]
                nc.scalar.activation(
                    out=st[:, 64 * hh:64 * hh + 64], in_=av[:, 0:HD],
                    func=mybir.ActivationFunctionType.Copy, scale=r[:])
                if qb == 0:
                    tp_tiles[p] = tpps.tile([128, 8, 128], BF16, tag="tp", name="tp", bufs=1)
                nc.tensor.transpose(tp_tiles[p][:, qb, :], in_=st[:],
                                    identity=ident[:])

            def ev_unit(p):
                """Evacuate pair p's 8 transposed blocks into outT[p]."""
                nc.vector.tensor_copy(
                    out=outT[p][:],
                    in_=tp_tiles[p][:].rearrange("p a b -> p (a b)"))

            outT = [outp.tile([128, NQ], BF16, tag="outT", name="outT_sb", bufs=PAIRS)
                    for _ in range(PAIRS)]

            # classifier partials: PA[qc, s0] = sum_{c<nch} outT[c] @ wf[c]
            PA = {}

            def pa_unit(qc, s0, nch=4):
                sw = min(512, NCLS - s0)
                ps = sps.tile([128, NQ], F32, tag="s", name="s", bufs=3)
                for c in range(nch):
                    nc.tensor.matmul(ps[:, 0:sw],
                                     lhsT=outT[c][:, qc * 128:(qc + 1) * 128],
                                     rhs=wf[:, c, s0:s0 + sw],
                                     start=(c == 0), stop=(c == nch - 1))
                pa = stp.tile([128, 512], BF16, tag="pa", name="pa", bufs=16)
                nc.vector.tensor_copy(out=pa[:, 0:sw], in_=ps[:, 0:sw])
                PA[(qc, s0)] = (pa, nch)

            # ---- schedule ----
            load_inputs_phase(0)
            alloc_qkP(0)
            qk_unit(0, "q", 0)
            qk_unit(0, "k", 0)
            load_inputs_phase(3)
            load_inputs_phase(2)
            load_inputs_phase(1)
            load_inputs_phase(4)
            make_identity(nc, ident)

            for h in range(HEADS):
                p = h // 2
                post = {}
                if h == 0:
                    # v production rides every slot; the key-half projection
                    # as late as its xTk dependency allows; pair-1
                    # projections mid-window.
                    for kc in range(10):
                        post.setdefault(kc, []).append(
                            lambda kc=kc: v_unit(kc))
                    post.setdefault(7, []).append(lambda: qk_unit(0, "k", 1024))
                    for (which, n0), kc in zip([("q", 0), ("k", 0)], [10, 12]):
                        post.setdefault(kc, []).append(
                            lambda which=which, n0=n0: qk_unit(1, which, n0))
                else:
                    # av of the previous head at odd slots; batched
                    # normalizes after each 4-block avt rotation.  Head 1
                    # defers its av chains so the remaining v production
                    # (kc 10-15) can finish first.
                    if h == 1:
                        for kc in range(10, KC):
                            post.setdefault(kc - 10, []).append(
                                lambda kc=kc: v_unit(kc))
                        for qb in range(8):
                            post.setdefault(6 + qb, []).append(
                                lambda h=h, qb=qb: av_mm(h - 1, qb))
                        post.setdefault(9, []).append(
                            lambda h=h: av_norm(h - 1, 0))
                        post.setdefault(15, []).append(
                            lambda h=h: av_norm(h - 1, 1))
                    else:
                        for qb in range(8):
                            post.setdefault(2 * qb + 1, []).append(
                                lambda h=h, qb=qb: av_mm(h - 1, qb))
                        post.setdefault(8, []).append(
                            lambda h=h: av_norm(h - 1, 0))
                        post.setdefault(15, []).append(
                            lambda h=h: av_norm(h - 1, 1))
                    if h % 2 == 0 and p + 1 < PAIRS:
                        for (which, n0), kc in zip([("q", 0), ("k", 0)], [5, 9]):
                            post.setdefault(kc, []).append(
                                lambda p=p, which=which, n0=n0: qk_unit(p + 1, which, n0))
                    if h % 2 == 1 and p + 1 < PAIRS:
                        post.setdefault(12, []).append(
                            lambda p=p: qk_unit(p + 1, "k", 1024))
                    if h == 9:
                        for i, qc in enumerate((0, 1, 2)):
                            post.setdefault(2 + 5 * i, []).append(
                                lambda qc=qc: pa_unit(qc, 0))
                            post.setdefault(4 + 5 * i, []).append(
                                lambda qc=qc: pa_unit(qc, 512))
                    if h == 10:
                        for i, qc in enumerate((3, 4, 5)):
                            post.setdefault(2 + 5 * i, []).append(
                                lambda qc=qc: pa_unit(qc, 0))
                            post.setdefault(4 + 5 * i, []).append(
                                lambda qc=qc: pa_unit(qc, 512))
                    if h == 11:
                        for i, qc in enumerate((6, 7)):
                            post.setdefault(5 + 6 * i, []).append(
                                lambda qc=qc: pa_unit(qc, 0, nch=5))
                            post.setdefault(8 + 6 * i, []).append(
                                lambda qc=qc: pa_unit(qc, 512, nch=5))
                # scores + exp stream for head h
                hh = h % 2
                es = []
                e_tiles[h] = es
                dve_kcs = DVE_KCS[h]
                for kc in range(KC):
                    j, sl = kc // 2, kc % 2
                    if sl == 0:
                        e2 = ep.tile([128, 2, NQ], FP8, tag="e", name="e", bufs=24)
                        es.append(e2)
                    s = sps.tile([128, NQ], F32, tag="s", name="s", bufs=3)
                    for n0 in range(0, NQ, 256):
                        nc.tensor.matmul(
                            s[:, n0:n0 + 256],
                            lhsT=kP[p][64 * hh:64 * hh + 64, :, kc * 128:(kc + 1) * 128],
                            rhs=qP[p][64 * hh:64 * hh + 64, :, n0:n0 + 256],
                            start=True, stop=True, perf_mode=DR)
                    if kc in dve_kcs:
                        nc.vector.tensor_scalar(
                            out=es[j][:, sl, :].bitcast(U8), in0=s[:],
                            scalar1=A_SCH, scalar2=B_SCH,
                            op0=mybir.AluOpType.mult, op1=mybir.AluOpType.add)
                    else:
                        nc.scalar.activation(out=es[j][:, sl, :], in_=s[:],
                                             func=EXP, scale=EXPSC)
                    for f in post.get(kc, ()):
                        f()
                if h >= 2 and h % 2 == 0:
                    tp_tiles[p - 1] = tpps.tile([128, 8, 128], BF16, tag="tp", name="tp", bufs=1)
                    for qb in range(8):
                        nc.tensor.transpose(tp_tiles[p - 1][:, qb, :],
                                            in_=st_tiles[p - 1][qb][:], identity=ident[:])
                    ev_unit(p - 1)

            # ---- tail: last head's attn@v + classifier finish + max ----
            lgmax = lgp.tile([128, NCLS], BF16, tag="lgmax")


            def cls_unit(qc):
                s = sps.tile([128, NQ], F32, tag="s", name="s", bufs=3)
                for s0 in (0, 512):
                    sw = min(512, NCLS - s0)
                    pa, nch = PA[(qc, s0)]
                    nc.tensor.matmul(s[:, s0:s0 + sw], lhsT=ident[:],
                                     rhs=pa[:, 0:sw],
                                     start=True, stop=False)
                    for c in range(nch, 6):
                        nc.tensor.matmul(s[:, s0:s0 + sw],
                                         lhsT=outT[c][:, qc * 128:(qc + 1) * 128],
                                         rhs=wf[:, c, s0:s0 + sw],
                                         start=False, stop=(c == 5))
                for s0 in (0, 512):
                    sw = min(512, NCLS - s0)
                    if qc == 0:
                        nc.vector.tensor_copy(out=lgmax[:, s0:s0 + sw],
                                              in_=s[:, s0:s0 + sw])
                    else:
                        nc.vector.tensor_max(out=lgmax[:, s0:s0 + sw],
                                             in0=s[:, s0:s0 + sw],
                                             in1=lgmax[:, s0:s0 + sw])

            def ev_qb(qb):
                nc.scalar.copy(
                    out=outT[5][:, qb * 128:(qb + 1) * 128],
                    in_=tp_tiles[5][:, qb, :])

            av_unit(11, 0)
            av_unit(11, 1)
            for qb in range(2, 8):
                ev_qb(qb - 2)
                av_unit(11, qb)
                cls_unit(qb - 2)
            ev_qb(6)
            cls_unit(6)
            ev_qb(7)
            cls_unit(7)

            nc.sync.dma_start(out=out_d[:, 0:512], in_=lgmax[:, 0:512])
            nc.sync.dma_start(out=out_d[:, 512:NCLS], in_=lgmax[:, 512:NCLS])

    nc.compile()
    return nc


def _prep_inputs(x, w_qkv, w_proj, b_proj, w_head, b_head):
    bf = ml_dtypes.bfloat16
    f8 = ml_dtypes.float8_e4m3
    x = np.asarray(x, dtype=np.float32)
    w_qkv = np.asarray(w_qkv, dtype=np.float32)
    wf = (np.asarray(w_proj, np.float64) @ np.asarray(w_head, np.float64))
    b_const = (np.asarray(b_proj, np.float32) @ np.asarray(w_head, np.float32)
               + np.asarray(b_head, np.float32))

    # DR row layout: row r of [C, X] -> (c, s, p) = (r//256, (r%256)//128, r%128)
    w8 = np.ascontiguousarray((w_qkv * WS).astype(f8))           # [768, 2304]
    wf_b = np.ascontiguousarray((wf / WS).astype(np.float32).astype(bf))
    in_maps = []
    for core in range(8):
        b, half = core // 2, core % 2
        xb = x[b] if half == 0 else np.concatenate(
            [x[b, NQ:], x[b, :NQ]], axis=0)   # rotate keys: own queries first
        xT8 = np.ascontiguousarray(xb.T.astype(f8))              # [768, 2048]
        qkp = np.ascontiguousarray(
            np.concatenate([xT8[:, :NQ], w8[:, 0:128], w8[:, C:C + 128]], axis=1))
        wrest = np.ascontiguousarray(
            np.concatenate([w8[:, 128:C], w8[:, C + 128:2 * C]], axis=1))
        xTk = np.ascontiguousarray(xT8[:, NQ:])
        wv = np.ascontiguousarray(w8[:, 2 * C:])
        in_maps.append({"qkp": qkp, "wrest": wrest, "xTk": xTk, "wv": wv,
                        "wf": wf_b})
    return in_maps, b_const


def kernel(x, w_qkv, w_proj, b_proj, w_head, b_head):
    if "nc" not in _CACHE:
        _CACHE["nc"] = _build()
    nc = _CACHE["nc"]

    in_maps, b_const = _prep_inputs(x, w_qkv, w_proj, b_proj, w_head, b_head)
    res = run_bass_kernel_spmd(nc, in_maps, core_ids=list(range(8)))

    out = np.empty((B, NUM_CLASSES), np.float32)
    for b in range(B):
        lo = res.results[2 * b]["out"].max(axis=0)
        hi = res.results[2 * b + 1]["out"].max(axis=0)
        out[b] = np.maximum(lo, hi)[:NUM_CLASSES] + b_const
    return out


if __name__ == "__main__":
    sys.path.insert(0, "/root/problem")
    import reference

    inputs = {k: np.asarray(v) for k, v in reference.setup_inputs().items()}
    expected = np.asarray(reference.reference(**inputs))
    actual = kernel(**inputs)
    num = np.linalg.norm(actual - expected)
    den = np.linalg.norm(expected)
    print("rel fro err:", num / den)


# revision 28
# speedup vs baseline: 1.0165x; 1.0165x over previous
"""Trainium2 Bass kernel for a ViT-style attention block + classifier head.

Reference computation (per batch b of 4, N=2048 tokens, C=768, 12 heads x 64):
    qkv  = x @ w_qkv                         [B,N,3C]
    attn = softmax(q k^T / 8)                per head
    out  = (attn @ v) reassembled            [B,N,C]
    out  = out @ w_proj + b_proj
    out  = out @ w_head + b_head             [B,N,1000]
    return max over N                        [B,1000]

Sharding: 8 cores = 4 batches x 2 query-halves (1024 queries each).
Each core computes K/V for its full batch, attention for its query half,
then a fused (w_proj @ w_head) classifier matmul and a local max over its
1024 queries -> [128,1000] per core; host reduces partitions + pairs and
adds the fused bias (max is invariant to per-row constants).

v3 design (vs the 242us v2 baseline) — attack all three busy engines:

* All projections (q/k/v) run in fp8e4m3 with MatmulPerfMode.DoubleRow:
  x and w_qkv are cast to fp8 host-side (w scaled by 32 to clear the
  subnormal range; compensated in the exp scale and classifier weight),
  packed in a [chunk, slot, partition] layout so each 256-deep
  contraction is 1 DR matmul (3 per 768 instead of 6 bf16 chunks).
  PE cost of the projections drops 4x (77us -> 19us).

* attn@v also runs DR: the exp stream writes fp8 e-tiles [128, 2, 1024]
  (kc-pair slots), v is produced once for all heads into [128, 2, 780]
  pair tiles (64 cols + ones col per head), so each (head, qb) output
  accumulates over 8 DR matmuls instead of 16 bf16 ones (41.6 -> 10.4us).
  fp8 perturbs only softmax weights and v; errors average across ~1.5k
  effective keys (measured end-to-end ~1.1e-2 vs the 2e-2 gate).

* The exp stream itself is split across TWO engines: ScalarE runs the
  Exp activation for ~10 kc of each head; the DVE runs a Schraudolph
  fast-exp for the other ~6 — one tensor_scalar per [128,1024] tile
  computing round(s * 8*log2e*scale + 55.55) into uint8, whose bits ARE
  fp8e4m3 exp(s*scale) to within +-8% (the piecewise-linear-in-mantissa
  exp approximation; bias cancels in softmax, noise averages out).
  HW CONSTRAINT (found by device bisection): an e2 tile whose two kc
  slots are written by DIFFERENT engines wedges the exec unit
  (NRT_EXEC_UNIT_UNRECOVERABLE), so DVE_KCS must hold whole kc-pairs.

* PSUM is one shared 3-deep ring of [128,1024]f32 tiles (6 banks) for
  scores AND the projection/v/pa evacuation slabs, + 1 bank for the
  4-rotation attn@v accumulator + 1 for the transpose stage.  The
  3-deep ring lets scores(kc+2) prefetch while the two exp engines
  drain kc and kc+1; batched 4-wide reciprocals, window-end-deferred
  transposes and a split lead-in DMA ([xTq|pair-0 w] first) keep the
  DVE queue from head-of-line blocking the stream.

* Classifier stays bf16 (fp8 dot-product noise does not average out
  there); split-partial (PA) + identity-refold tail, local max on the
  idle scores ring.

Measured (TimelineSim on the exact per-core program): 201.2 us/core
(v2 baseline: 241.8), rel err vs the fp32 reference 1.05e-2 on real
TRN2 via the axon relay.  ScalarE ~137us busy, DVE ~140us busy are the
co-critical engines; PE ~100us.
"""

import math
import sys

for _p in ("/opt/trn_rl_repo", "/root/.axon_site/_ro/trn_rl_repo"):
    if _p not in sys.path:
        sys.path.append(_p)

import numpy as np
import ml_dtypes

import concourse.bacc as bacc
import concourse.mybir as mybir
from concourse.tile import TileContext
from concourse.bass_utils import run_bass_kernel_spmd
from concourse.masks import make_identity

BF16 = mybir.dt.bfloat16
F32 = mybir.dt.float32
FP8 = mybir.dt.float8e4
U8 = mybir.dt.uint8
DR = mybir.MatmulPerfMode.DoubleRow

B, N, C = 4, 2048, 768
HEADS, HD = 12, 64
NUM_CLASSES = 1000
SCALE = HD ** (-0.5)
WS = 32.0                    # host-side fp8 weight scale
EXPSC = SCALE / (WS * WS)    # exp input scale (q,k each carry a WS)

NQ = 1024           # queries per core
KC = N // 128       # 16 key chunks
CC = 3              # DR contraction chunks (256 rows each)
PAIRS = HEADS // 2
NCLS = NUM_CLASSES
NJ = KC // 2        # 8 kc-pairs (DR attn@v contraction steps)

# Schraudolph fast-exp constants: uint8 bits = round(s*A_SCH + B_SCH)
# reinterpreted as fp8e4m3 ~= exp(s*EXPSC).
A_SCH = EXPSC * 8.0 / math.log(2.0)
B_SCH = 55.55

# kc values whose exp runs on the DVE (per head); the rest run on ScalarE.
# Isolated (non-adjacent) kc keep the 2-bank scores ring from coupling the
# two exp streams: Act only waits out the ~0.2us rate difference, not a
# whole DVE unit.
# PAIR-granular only: an e2 tile written by both engines (one slot each)
# crashes the exec unit on hardware — every kc-pair's two slots must come
# from ONE engine.
# kc -> (e2/v65 tile, slot) pairing.  The pairing is arbitrary as long as
# the exp stream and the v production agree; pairing NON-adjacent kc into
# the DVE tiles keeps each DVE exp time-isolated (one scores-ring slot at
# a time) while still giving every tile a single writing engine.
PAIRING = [(0, 1), (2, 5), (3, 4), (6, 7), (8, 11), (9, 10), (12, 14), (13, 15)]
TILE_OF = {}
SLOT_OF = {}
for _j, (_a, _b) in enumerate(PAIRING):
    TILE_OF[_a] = _j; SLOT_OF[_a] = 0
    TILE_OF[_b] = _j; SLOT_OF[_b] = 1
DVE_KCS = {h: ((2, 5, 8, 11, 13, 15) if h >= 1 else ()) for h in range(HEADS)}
DVE_KCS[1] = (8, 11, 13, 15)   # head 1's DVE is busy with the v-production

_CACHE = {}


def _build():
    nc = bacc.Bacc("TRN2", target_bir_lowering=False)

    # Host-packed fp8 inputs in DR row layout (row = c*256 + s*128 + p):
    #   qkp: [xT query half | w_q | w_k] per row — the lead-in working set
    #   xTk: key half of xT;  wv: w_v;  wf: fused classifier weight (bf16,
    #   plain c*128+p rows).
    qkp_d = nc.dram_tensor("qkp", [2 * CC * 128, NQ + 256], FP8, kind="ExternalInput")
    wrest_d = nc.dram_tensor("wrest", [2 * CC * 128, 2 * C - 256], FP8, kind="ExternalInput")
    xTk_d = nc.dram_tensor("xTk", [2 * CC * 128, NQ], FP8, kind="ExternalInput")
    wv_d = nc.dram_tensor("wv", [2 * CC * 128, C], FP8, kind="ExternalInput")
    wf_d = nc.dram_tensor("wf", [C, NCLS], BF16, kind="ExternalInput")
    out_d = nc.dram_tensor("out", [128, NCLS], BF16, kind="ExternalOutput")

    EXP = mybir.ActivationFunctionType.Exp

    with TileContext(nc) as tc:
        with (
            tc.tile_pool(name="wpool", bufs=1) as wpool,
            tc.tile_pool(name="xpool", bufs=1) as xpool,
            tc.tile_pool(name="stgp", bufs=1) as stgp,    # fp8 q/k DR tiles
            tc.tile_pool(name="vp", bufs=1) as vp,        # v65 pair tiles
            tc.tile_pool(name="ep", bufs=1) as ep,        # fp8 e2 tiles (2 head-sets)
            tc.tile_pool(name="stp", bufs=1) as stp,      # normalized [q, 2hd] staging
            tc.tile_pool(name="outp", bufs=1) as outp,
            tc.tile_pool(name="smallp", bufs=1) as smallp,
            tc.tile_pool(name="lgp", bufs=1) as lgp,
            # PSUM: one shared 3-deep ring of [128,1024]f32 tiles (6 banks)
            # serving scores AND the proj/v/pa evacuation slabs + av ring (1)
            # + transpose stage (1).  3-deep decouples the two exp engines:
            # scores(kc+2) prefetches while exp(kc)/exp(kc+1) drain.
            tc.tile_pool(name="sps", bufs=1, space="PSUM") as sps,
            tc.tile_pool(name="avps", bufs=1, space="PSUM") as avps,
            tc.tile_pool(name="tpps", bufs=1, space="PSUM") as tpps,
        ):
            ident = smallp.tile([128, 128], BF16, name="ident")

            # ---- persistent inputs ----
            xw = xpool.tile([128, CC, 2, NQ + 256], FP8, tag="xw", name="xw_sb")
            wrest = xpool.tile([128, CC, 2, 2 * C - 256], FP8, tag="wrest", name="wrest_sb")
            xTk = xpool.tile([128, CC, 2, NQ], FP8, tag="xTk", name="xTk_sb")
            wv_sb = wpool.tile([128, CC, 2, C], FP8, tag="wv", name="wv_sb")
            wf = wpool.tile([128, 2 * CC, NCLS], BF16, tag="wf", name="wf_sb")

            def xslice(c, n0, nw):
                """[128, 2, nw] DR chunk of xT columns [n0, n0+nw)."""
                assert n0 // NQ == (n0 + nw - 1) // NQ
                if n0 < NQ:
                    return xw[:, c, :, n0:n0 + nw]
                return xTk[:, c, :, n0 - NQ:n0 - NQ + nw]

            def wcol(c, which, p):
                """DR weight block [128, 2, 128] for pair p's q or k."""
                if p == 0:
                    o = NQ + (0 if which == "q" else 128)
                    return xw[:, c, :, o:o + 128]
                o = (p - 1) * 128 + (0 if which == "q" else 5 * 128)
                return wrest[:, c, :, o:o + 128]

            def load_inputs_phase(phase):
                if phase == 0:
                    # qkp in (c, s) row groups of [2,1,2,1] alternating the
                    # two HW queues; DR matmul c needs groups up to 2c+1 so
                    # the projection pipeline starts after 1/3 of the bytes.
                    o = 0
                    for i, g in enumerate((2, 1, 2, 1)):
                        eng = nc.sync if i % 2 == 0 else nc.scalar
                        eng.dma_start(
                            out=xw[:].rearrange("p a b n -> p (a b) n")[:, o:o + g, :],
                            in_=qkp_d[o * 128:(o + g) * 128, :].rearrange(
                                "(a p) n -> p a n", p=128))
                        o += g
                elif phase == 1:        # weights for pairs 1-5
                    nc.scalar.dma_start(
                        out=wrest[:].rearrange("p a b n -> p (a b) n"),
                        in_=wrest_d[:].rearrange("(a p) n -> p a n", p=128))
                elif phase == 2:        # key half of xT
                    nc.scalar.dma_start(
                        out=xTk[:].rearrange("p a b n -> p (a b) n"),
                        in_=xTk_d[:].rearrange("(a p) n -> p a n", p=128))
                elif phase == 3:        # w_v
                    nc.scalar.dma_start(
                        out=wv_sb[:].rearrange("p a b n -> p (a b) n"),
                        in_=wv_d[:].rearrange("(a p) n -> p a n", p=128))
                elif phase == 4:        # classifier weight (tail only)
                    nc.scalar.dma_start(
                        out=wf[:],
                        in_=wf_d[:].rearrange("(a p) n -> p a n", p=128))

            # fp8 q/k per pair in scores-DR layout [128, 2, N]: slot 0 holds
            # the real qT/kT (pair's two heads stacked on partitions, exactly
            # the projection PSUM layout -> lane-local cast), slot 1 zeroed.
            qP = {}
            kP = {}

            def alloc_qkP(p):
                if p not in qP:
                    qP[p] = stgp.tile([128, 2, NQ], FP8, tag="qP", name="qP_sb", bufs=2)
                    nc.gpsimd.memset(qP[p][:, 1, :], 0.0)
                if p not in kP:
                    kP[p] = stgp.tile([128, 2, N], FP8, tag="kP", name="kP_sb", bufs=2)
                    nc.gpsimd.memset(kP[p][:, 1, :], 0.0)

            def qk_unit(p, which, n0, nw=1024):
                """One 1024-col slab of pair p's q or k projection: 3 DR
                matmuls (bank-aligned 512 halves) -> lane-local fp8 cast
                into the scores-DR tile."""
                alloc_qkP(p)
                dst = qP[p] if which == "q" else kP[p]
                ps = sps.tile([128, NQ], F32, tag="s", name="s", bufs=3)
                for c in range(CC):
                    for o in range(0, nw, 512):
                        nc.tensor.matmul(
                            ps[:, o:o + 512], lhsT=wcol(c, which, p),
                            rhs=xslice(c, n0 + o, 512),
                            start=(c == 0), stop=(c == CC - 1), perf_mode=DR)
                if p == 0 and which == "k" and n0 < NQ:
                    # pair 0's first k cast rides the still-idle Activation
                    # queue so the lead-in cast chain runs two-wide
                    nc.scalar.copy(out=dst[:, 0, n0:n0 + nw], in_=ps[:, 0:nw])
                else:
                    nc.vector.tensor_copy(out=dst[:, 0, n0:n0 + nw], in_=ps[:, 0:nw])

            # v65 pair tiles: v65[j][:, s, :] holds keys of kc=2j+s, columns
            # h*65..h*65+64 = head h's v (+ ones col at h*65+64).
            v65 = [vp.tile([128, 2, HEADS * (HD + 1)], FP8, tag="v65",
                           name="v65_sb", bufs=NJ)
                   for _ in range(NJ)]
            v65_ones = [False] * NJ

            def v_unit(kc):
                """v for keys of chunk kc, all 12 heads (bank-aligned
                512/256 matmul halves, one strided evacuation)."""
                j, s = TILE_OF[kc], SLOT_OF[kc]
                ps = sps.tile([128, NQ], F32, tag="s", name="s", bufs=3)
                for c in range(CC):
                    for o, w in ((0, 512), (512, 256)):
                        nc.tensor.matmul(
                            ps[:, o:o + w], lhsT=xslice(c, kc * 128, 128),
                            rhs=wv_sb[:, c, :, o:o + w],
                            start=(c == 0), stop=(c == CC - 1), perf_mode=DR)
                vdst = v65[j][:, s, :].rearrange("p (h d) -> p h d", d=HD + 1)
                if not v65_ones[j]:
                    v65_ones[j] = True
                    od = v65[j][:].rearrange("p s (h d) -> p s h d", d=HD + 1)
                    nc.gpsimd.memset(od[:, :, :, HD:HD + 1], 1.0)
                nc.vector.tensor_copy(
                    out=vdst[:, :, 0:HD],
                    in_=ps[:, 0:C].rearrange("p (h d) -> p h d", d=HD))

            e_tiles = {}      # h -> [128, 2, NQ] fp8 tile list per kc-pair j
            st_tiles = {}     # p -> [8 staging tiles]
            tp_tiles = {}
            tail_r = {}

            def av_mm(h, qb):
                """attn@v matmul chain for head h, query block qb:
                out[q,65] accumulated over 8 DR kc-pair matmuls."""
                if "av" not in tp_tiles:
                    tp_tiles["av"] = avps.tile([128, 4, HD + 1], F32, name="avt", bufs=1)
                av = tp_tiles["av"][:, qb % 4, :]
                es = e_tiles[h]
                for i, (j, et) in enumerate(es):
                    nc.tensor.matmul(
                        av[:], lhsT=et[:, :, qb * 128:(qb + 1) * 128],
                        rhs=v65[j][:, :, h * (HD + 1):(h + 1) * (HD + 1)],
                        start=(i == 0), stop=(i == NJ - 1), perf_mode=DR)

            def av_norm(h, g):
                """normalize query blocks 4g..4g+3 of head h: one batched
                4-wide reciprocal over the avt rotation, then 4 scaled
                evacuations into the transpose staging tiles."""
                p, hh = h // 2, h % 2
                avt = tp_tiles["av"]
                r4 = smallp.tile([128, 4], F32, tag="r", name="r", bufs=2)
                nc.vector.reciprocal_approx_fast(out=r4[:], in_=avt[:, :, HD])
                for qb in range(4 * g, 4 * g + 4):
                    av = avt[:, qb % 4, :]
                    if hh == 0:
                        if p not in st_tiles:
                            st_tiles[p] = []
                        st = stp.tile([128, 128], BF16, tag="st", name="st", bufs=16)
                        st_tiles[p].append(st)
                    else:
                        st = st_tiles[p][qb]
                    nc.vector.tensor_scalar_mul(
                        out=st[:, 64 * hh:64 * hh + 64], in0=av[:, 0:HD],
                        scalar1=r4[:, qb % 4:qb % 4 + 1])

            def av_unit(h, qb):
                """tail-only (head 11): av chain + per-qb normalize fully
                on the then-idle ScalarE (one-time switch to the
                reciprocal act table keeps the DVE out of the chain; it
                only runs the final maxes)."""
                p, hh = h // 2, h % 2
                av_mm(h, qb)
                av = tp_tiles["av"][:, qb % 4, :]
                r = smallp.tile([128, 1], F32, tag="r1", name="r1", bufs=4)
                nc.vector.reciprocal_approx_fast(out=r[:], in_=av[:, HD:HD + 1])
                st = st_tiles[p][qb]
                nc.scalar.activation(
                    out=st[:, 64 * hh:64 * hh + 64], in_=av[:, 0:HD],
                    func=mybir.ActivationFunctionType.Copy, scale=r[:])
                if qb == 0:
                    tp_tiles[p] = tpps.tile([128, 8, 128], BF16, tag="tp", name="tp", bufs=1)
                nc.tensor.transpose(tp_tiles[p][:, qb, :], in_=st[:],
                                    identity=ident[:])

            def ev_unit(p):
                """Evacuate pair p's 8 transposed blocks into outT[p]."""
                nc.vector.tensor_copy(
                    out=outT[p][:],
                    in_=tp_tiles[p][:].rearrange("p a b -> p (a b)"))

            outT = [outp.tile([128, NQ], BF16, tag="outT", name="outT_sb", bufs=PAIRS)
                    for _ in range(PAIRS)]

            # classifier partials: PA[qc, s0] = sum_{c<nch} outT[c] @ wf[c]
            PA = {}

            def pa_unit(qc, s0, nch=4):
                sw = min(512, NCLS - s0)
                ps = sps.tile([128, NQ], F32, tag="s", name="s", bufs=3)
                for c in range(nch):
                    nc.tensor.matmul(ps[:, 0:sw],
                                     lhsT=outT[c][:, qc * 128:(qc + 1) * 128],
                                     rhs=wf[:, c, s0:s0 + sw],
                                     start=(c == 0), stop=(c == nch - 1))
                pa = stp.tile([128, 512], BF16, tag="pa", name="pa", bufs=16)
                nc.vector.tensor_copy(out=pa[:, 0:sw], in_=ps[:, 0:sw])
                PA[(qc, s0)] = (pa, nch)

            # ---- schedule ----
            load_inputs_phase(0)
            alloc_qkP(0)
            qk_unit(0, "q", 0)
            qk_unit(0, "k", 0)
            load_inputs_phase(3)
            load_inputs_phase(2)
            load_inputs_phase(1)
            load_inputs_phase(4)
            make_identity(nc, ident)

            for h in range(HEADS):
                p = h // 2
                post = {}
                if h == 0:
                    # v production rides every slot; the key-half projection
                    # as late as its xTk dependency allows; pair-1
                    # projections mid-window.
                    for kc in range(10):
                        post.setdefault(kc, []).append(
                            lambda kc=kc: v_unit(kc))
                    post.setdefault(7, []).append(lambda: qk_unit(0, "k", 1024))
                    for (which, n0), kc in zip([("q", 0), ("k", 0)], [10, 12]):
                        post.setdefault(kc, []).append(
                            lambda which=which, n0=n0: qk_unit(1, which, n0))
                else:
                    if h % 2 == 1 and h >= 3:
                        # pair (h//2 - 1)'s transposes + evacuation ride
                        # post[0] so they can't delay this head's first
                        # scores at the boundary
                        def tp_ev(pp):
                            tp_tiles[pp] = tpps.tile([128, 8, 128], BF16,
                                                     tag="tp", name="tp", bufs=1)
                            for qb in range(8):
                                nc.tensor.transpose(tp_tiles[pp][:, qb, :],
                                                    in_=st_tiles[pp][qb][:],
                                                    identity=ident[:])
                            ev_unit(pp)
                        post.setdefault(0, []).append(
                            lambda pp=h // 2 - 1: tp_ev(pp))
                    # av of the previous head at odd slots; batched
                    # normalizes after each 4-block avt rotation.  Head 1
                    # defers its av chains so the remaining v production
                    # (kc 10-15) can finish first.
                    if h == 1:
                        for kc in range(10, KC):
                            post.setdefault(kc - 10, []).append(
                                lambda kc=kc: v_unit(kc))
                        for qb in range(8):
                            post.setdefault(6 + qb, []).append(
                                lambda h=h, qb=qb: av_mm(h - 1, qb))
                        post.setdefault(9, []).append(
                            lambda h=h: av_norm(h - 1, 0))
                        post.setdefault(15, []).append(
                            lambda h=h: av_norm(h - 1, 1))
                    else:
                        for qb in range(8):
                            post.setdefault(2 * qb + 1, []).append(
                                lambda h=h, qb=qb: av_mm(h - 1, qb))
                        post.setdefault(8, []).append(
                            lambda h=h: av_norm(h - 1, 0))
                        post.setdefault(15, []).append(
                            lambda h=h: av_norm(h - 1, 1))
                    if h % 2 == 0 and p + 1 < PAIRS:
                        for (which, n0), kc in zip([("q", 0), ("k", 0)], [5, 9]):
                            post.setdefault(kc, []).append(
                                lambda p=p, which=which, n0=n0: qk_unit(p + 1, which, n0))
                    if h % 2 == 1 and p + 1 < PAIRS:
                        post.setdefault(12, []).append(
                            lambda p=p: qk_unit(p + 1, "k", 1024))
                    if h == 9:
                        for i, qc in enumerate((0, 1, 2)):
                            post.setdefault(2 + 5 * i, []).append(
                                lambda qc=qc: pa_unit(qc, 0))
                            post.setdefault(4 + 5 * i, []).append(
                                lambda qc=qc: pa_unit(qc, 512))
                    if h == 10:
                        for i, qc in enumerate((3, 4, 5)):
                            post.setdefault(2 + 5 * i, []).append(
                                lambda qc=qc: pa_unit(qc, 0))
                            post.setdefault(4 + 5 * i, []).append(
                                lambda qc=qc: pa_unit(qc, 512))
                    if h == 11:
                        for i, qc in enumerate((6, 7)):
                            post.setdefault(5 + 6 * i, []).append(
                                lambda qc=qc: pa_unit(qc, 0, nch=5))
                            post.setdefault(8 + 6 * i, []).append(
                                lambda qc=qc: pa_unit(qc, 512, nch=5))
                # scores + exp stream for head h
                hh = h % 2
                es = []
                e_tiles[h] = es
                dve_kcs = DVE_KCS[h]
                etile = {}
                for kc in range(KC):
                    j, sl = TILE_OF[kc], SLOT_OF[kc]
                    if j not in etile:
                        e2 = ep.tile([128, 2, NQ], FP8, tag="e", name="e", bufs=24)
                        etile[j] = e2
                        es.append((j, e2))
                    s = sps.tile([128, NQ], F32, tag="s", name="s", bufs=3)
                    for n0 in range(0, NQ, 256):
                        nc.tensor.matmul(
                            s[:, n0:n0 + 256],
                            lhsT=kP[p][64 * hh:64 * hh + 64, :, kc * 128:(kc + 1) * 128],
                            rhs=qP[p][64 * hh:64 * hh + 64, :, n0:n0 + 256],
                            start=True, stop=True, perf_mode=DR)
                    if kc in dve_kcs:
                        nc.vector.tensor_scalar(
                            out=etile[j][:, sl, :].bitcast(U8), in0=s[:],
                            scalar1=A_SCH, scalar2=B_SCH,
                            op0=mybir.AluOpType.mult, op1=mybir.AluOpType.add)
                    else:
                        nc.scalar.activation(out=etile[j][:, sl, :], in_=s[:],
                                             func=EXP, scale=EXPSC)
                    for f in post.get(kc, ()):
                        f()


            # ---- tail: last head's attn@v + classifier finish + max ----
            lgmax = lgp.tile([128, NCLS], BF16, tag="lgmax")


            def cls_unit(qc):
                s = sps.tile([128, NQ], F32, tag="s", name="s", bufs=3)
                for s0 in (0, 512):
                    sw = min(512, NCLS - s0)
                    pa, nch = PA[(qc, s0)]
                    nc.tensor.matmul(s[:, s0:s0 + sw], lhsT=ident[:],
                                     rhs=pa[:, 0:sw],
                                     start=True, stop=False)
                    for c in range(nch, 6):
                        nc.tensor.matmul(s[:, s0:s0 + sw],
                                         lhsT=outT[c][:, qc * 128:(qc + 1) * 128],
                                         rhs=wf[:, c, s0:s0 + sw],
                                         start=False, stop=(c == 5))
                for s0 in (0, 512):
                    sw = min(512, NCLS - s0)
                    if qc == 0:
                        nc.vector.tensor_copy(out=lgmax[:, s0:s0 + sw],
                                              in_=s[:, s0:s0 + sw])
                    else:
                        nc.vector.tensor_max(out=lgmax[:, s0:s0 + sw],
                                             in0=s[:, s0:s0 + sw],
                                             in1=lgmax[:, s0:s0 + sw])

            def ev_qb(qb):
                nc.scalar.copy(
                    out=outT[5][:, qb * 128:(qb + 1) * 128],
                    in_=tp_tiles[5][:, qb, :])

            av_unit(11, 0)
            av_unit(11, 1)
            for qb in range(2, 8):
                ev_qb(qb - 2)
                av_unit(11, qb)
                cls_unit(qb - 2)
            ev_qb(6)
            cls_unit(6)
            ev_qb(7)
            cls_unit(7)

            nc.sync.dma_start(out=out_d[:, 0:512], in_=lgmax[:, 0:512])
            nc.sync.dma_start(out=out_d[:, 512:NCLS], in_=lgmax[:, 512:NCLS])

    nc.compile()
    return nc


def _prep_inputs(x, w_qkv, w_proj, b_proj, w_head, b_head):
    bf = ml_dtypes.bfloat16
    f8 = ml_dtypes.float8_e4m3
    x = np.asarray(x, dtype=np.float32)
    w_qkv = np.asarray(w_qkv, dtype=np.float32)
    wf = (np.asarray(w_proj, np.float64) @ np.asarray(w_head, np.float64))
    b_const = (np.asarray(b_proj, np.float32) @ np.asarray(w_head, np.float32)
               + np.asarray(b_head, np.float32))

    # DR row layout: row r of [C, X] -> (c, s, p) = (r//256, (r%256)//128, r%128)
    w8 = np.ascontiguousarray((w_qkv * WS).astype(f8))           # [768, 2304]
    wf_b = np.ascontiguousarray((wf / WS).astype(np.float32).astype(bf))
    in_maps = []
    for core in range(8):
        b, half = core // 2, core % 2
        xb = x[b] if half == 0 else np.concatenate(
            [x[b, NQ:], x[b, :NQ]], axis=0)   # rotate keys: own queries first
        xT8 = np.ascontiguousarray(xb.T.astype(f8))              # [768, 2048]
        qkp = np.ascontiguousarray(
            np.concatenate([xT8[:, :NQ], w8[:, 0:128], w8[:, C:C + 128]], axis=1))
        wrest = np.ascontiguousarray(
            np.concatenate([w8[:, 128:C], w8[:, C + 128:2 * C]], axis=1))
        xTk = np.ascontiguousarray(xT8[:, NQ:])
        wv = np.ascontiguousarray(w8[:, 2 * C:])
        in_maps.append({"qkp": qkp, "wrest": wrest, "xTk": xTk, "wv": wv,
                        "wf": wf_b})
    return in_maps, b_const


def kernel(x, w_qkv, w_proj, b_proj, w_head, b_head):
    if "nc" not in _CACHE:
        _CACHE["nc"] = _build()
    nc = _CACHE["nc"]

    in_maps, b_const = _prep_inputs(x, w_qkv, w_proj, b_proj, w_head, b_head)
    res = run_bass_kernel_spmd(nc, in_maps, core_ids=list(range(8)))

    out = np.empty((B, NUM_CLASSES), np.float32)
    for b in range(B):
        lo = res.results[2 * b]["out"].max(axis=0)
        hi = res.results[2 * b + 1]["out"].max(axis=0)
        out[b] = np.maximum(lo, hi)[:NUM_CLASSES] + b_const
    return out


if __name__ == "__main__":
    sys.path.insert(0, "/root/problem")
    import reference

    inputs = {k: np.asarray(v) for k, v in reference.setup_inputs().items()}
    expected = np.asarray(reference.reference(**inputs))
    actual = kernel(**inputs)
    num = np.linalg.norm(actual - expected)
    den = np.linalg.norm(expected)
    print("rel fro err:", num / den)
